# revision 1
# baseline (speedup 1.0000x reference)
"""Trainium2 Bass kernel for a 2-layer GCN graph classifier.

Strategy (pure data parallelism over graphs, per sharding hint):
  - Graphs are partitioned into 8 contiguous groups (batch vector is sorted),
    nodes/edges follow.  Each core owns the edges whose *dst* falls in its
    node range (plus self-loops).
  - Math restructure: with norm_e = dinv[src]*dinv[dst] folded per-edge into
    the one-hot selection matrix, segment-sum aggregation becomes plain
    matmuls on the TensorEngine:
        aggT[h, d] = sum_chunks  msg_chunk[e,h].T @ MT_chunk[e,d]
    where MT[e,d] = (dstl_e == d) * norm_e is built in ONE fused DVE op
    (tensor_scalar is_equal + mult) per 128-edge chunk.
  - Layer 1 gathers rows of the small (embed @ W1) table [5120,128] (indices
    pre-composed on host: idx = node_ids[src]); layer 2 gathers rows of the
    exchanged h2 table.  Gathers are batched indirect DMAs (SWDGE).
  - Two launches:  AB = build embed@W1 + layer-1 + h2 tables (per-core
    output); host concatenates h2 slices; C = layer-2 + mean-pool + head.
  - fp16 operands, fp32 PSUM accumulation.
"""

import sys

sys.path.insert(0, "/opt/trn_rl_repo")

import numpy as np

import concourse.bacc as bacc
import concourse.bass as bass
import concourse.mybir as mybir
import concourse.tile as tile
from concourse.bass import IndirectOffsetOnAxis

P = 128
NCORES = 8
F16 = mybir.dt.float16
F32 = mybir.dt.float32
I32 = mybir.dt.int32
AF = mybir.ActivationFunctionType
OP = mybir.AluOpType

EMB = 64
HID = 128
NCLS = 16
SBN = 8  # blocks per gather superblock


def _ceil(a, b):
    return -(-a // b)


# ---------------------------------------------------------------- host prep


def _prep(node_ids, edge_index, batch, n_graphs, vocab):
    N = node_ids.shape[0]
    src = np.asarray(edge_index[0], np.int64)
    dst = np.asarray(edge_index[1], np.int64)
    batch = np.asarray(batch, np.int64)
    node_ids = np.asarray(node_ids, np.int64)
    Gpc = n_graphs // NCORES
    cuts = np.searchsorted(batch, np.arange(NCORES + 1) * Gpc)
    deg = (np.bincount(dst, minlength=N) + 1).astype(np.float64)
    L = cuts[1:] - cuts[:-1]
    NB = int(max(_ceil(int(l), P) for l in L))
    Lpad = NB * P
    slot_of = np.empty(N, np.int64)
    for c in range(NCORES):
        slot_of[cuts[c]:cuts[c + 1]] = c * Lpad + np.arange(cuts[c + 1] - cuts[c])

    dstcore = np.searchsorted(cuts[1:], dst, side="right")
    percore = []
    K = 0
    GB = _ceil(Gpc, P)
    K_pool = 0
    for c in range(NCORES):
        m = dstcore == c
        es = np.concatenate([src[m], np.arange(cuts[c], cuts[c + 1])])
        ed = np.concatenate([dst[m], np.arange(cuts[c], cuts[c + 1])])
        bid = (ed - cuts[c]) >> 7
        o = np.argsort(bid, kind="stable")
        es, ed, bid = es[o], ed[o], bid[o]
        cnts = np.bincount(bid, minlength=NB)
        K = max(K, int(_ceil(int(cnts.max()), P)))
        gl = batch[cuts[c]:cuts[c + 1]] - c * Gpc
        gb = gl >> 7
        gcnts = np.bincount(gb, minlength=GB)
        K_pool = max(K_pool, int(_ceil(int(gcnts.max()), P)))
        percore.append((es, ed, bid, cnts, gl, gb, gcnts))

    cores = []
    for c in range(NCORES):
        es, ed, bid, cnts, gl, gb, gcnts = percore[c]
        start = np.zeros(NB, np.int64)
        start[1:] = np.cumsum(cnts)[:-1]
        rank = np.arange(len(es)) - start[bid]
        jg = bid * K + (rank >> 7)
        pp = rank & 127
        J = NB * K
        idx1 = np.zeros((P, J), np.int32)
        idx2 = np.zeros((P, J), np.int32)
        dstl = np.full((P, J), -1.0, np.float32)
        degs = np.ones((P, J), np.float16)
        degd = np.ones((P, J), np.float16)
        idx1[pp, jg] = node_ids[es].astype(np.int32)
        idx2[pp, jg] = slot_of[es].astype(np.int32)
        dstl[pp, jg] = (ed - cuts[c] - (bid << 7)).astype(np.float32)
        degs[pp, jg] = deg[es].astype(np.float16)
        degd[pp, jg] = deg[ed].astype(np.float16)

        Lc = cuts[c + 1] - cuts[c]
        gstart = np.zeros(GB, np.int64)
        gstart[1:] = np.cumsum(gcnts)[:-1]
        r = np.arange(Lc) - gstart[gb]
        jq = gb * K_pool + (r >> 7)
        pq = r & 127
        Jp = GB * K_pool
        poolidx = np.zeros((P, Jp), np.int32)
        batchrel = np.full((P, Jp), -1.0, np.float32)
        poolidx[pq, jq] = np.arange(Lc, dtype=np.int32)
        batchrel[pq, jq] = (gl - (gb << 7)).astype(np.float32)
        cores.append(dict(idx1=idx1, idx2=idx2, dstl=dstl, degs=degs, degd=degd,
                          poolidx=poolidx, batchrel=batchrel))
    meta = dict(NB=NB, K=K, GB=GB, K_pool=K_pool, Lpad=Lpad, Gpc=Gpc,
                Vpad=_ceil(vocab, P) * P)
    return cores, meta


# ------------------------------------------------------------ program builders


def _edge_layer(nc, tc, ctx, NB, K, table_ap, idx_d, dstl_d, degs_d, degd_d,
                iota_sb, bias_sb, W2_sb, h2_out, ident_sb, x3_tile):
    """Shared edge-aggregation pipeline.  If W2_sb is not None -> layer 1
    (x2T @ W2 -> h2 rows to h2_out dram).  Else layer 2 -> transpose x3T and
    store node-major rows into x3_tile (DRAM tile)."""
    idx_p = ctx.enter_context(tc.tile_pool(name="idxp", bufs=2))
    msg_p = ctx.enter_context(tc.tile_pool(name="msgp", bufs=2))
    nrm_p = ctx.enter_context(tc.tile_pool(name="nrmp", bufs=2))
    mt_p = ctx.enter_context(tc.tile_pool(name="mtp", bufs=4))
    xo_p = ctx.enter_context(tc.tile_pool(name="xop", bufs=3))
    agg_p = ctx.enter_context(tc.tile_pool(name="aggps", bufs=2, space="PSUM"))
    h2_p = ctx.enter_context(tc.tile_pool(name="h2ps", bufs=2, space="PSUM"))

    NSB = _ceil(NB, SBN)
    for sb in range(NSB):
        b0 = sb * SBN
        nb = min(SBN, NB - b0)
        Js = nb * K
        j0 = b0 * K
        idx_t = idx_p.tile([P, Js], I32, tag="idx")
        nc.sync.dma_start(idx_t[:, :], idx_d[:, j0:j0 + Js])
        dstl_t = idx_p.tile([P, Js], F32, tag="dstl")
        nc.sync.dma_start(dstl_t[:, :], dstl_d[:, j0:j0 + Js])
        degs_t = idx_p.tile([P, Js], F16, tag="degs")
        nc.sync.dma_start(degs_t[:, :], degs_d[:, j0:j0 + Js])
        degd_t = idx_p.tile([P, Js], F16, tag="degd")
        nc.sync.dma_start(degd_t[:, :], degd_d[:, j0:j0 + Js])

        msg_t = msg_p.tile([P, Js * P], F16, tag="msg")
        for j in range(Js):
            nc.gpsimd.indirect_dma_start(
                out=msg_t[:, j * P:(j + 1) * P], out_offset=None, in_=table_ap,
                in_offset=IndirectOffsetOnAxis(ap=idx_t[:, j:j + 1], axis=0))

        sq_s = nrm_p.tile([P, Js], F32, tag="sqs")
        nc.scalar.activation(sq_s[:, :], degs_t[:, :], AF.Sqrt)
        sq_d = nrm_p.tile([P, Js], F32, tag="sqd")
        nc.scalar.activation(sq_d[:, :], degd_t[:, :], AF.Sqrt)
        prod = nrm_p.tile([P, Js], F32, tag="prod")
        nc.vector.tensor_tensor(out=prod[:, :], in0=sq_s[:, :], in1=sq_d[:, :],
                                op=OP.mult)
        normf = nrm_p.tile([P, Js], F32, tag="normf")
        nc.vector.reciprocal(normf[:, :], prod[:, :])

        for bi in range(nb):
            b = b0 + bi
            agg = agg_p.tile([P, P], F32, tag="agg")
            for k in range(K):
                j = bi * K + k
                mt = mt_p.tile([P, P], F16, tag="mt")
                nc.vector.tensor_scalar(
                    out=mt[:, :], in0=iota_sb[:, :],
                    scalar1=dstl_t[:, j:j + 1], scalar2=normf[:, j:j + 1],
                    op0=OP.is_equal, op1=OP.mult)
                nc.tensor.matmul(agg[:, :], lhsT=msg_t[:, j * P:(j + 1) * P],
                                 rhs=mt[:, :], start=(k == 0), stop=(k == K - 1))
            xT = xo_p.tile([P, P], F16, tag="xT")
            nc.scalar.activation(xT[:, :], agg[:, :], AF.Relu, bias=bias_sb[:, :])
            if W2_sb is not None:
                h2ps = h2_p.tile([P, P], F32, tag="h2ps")
                nc.tensor.matmul(h2ps[:, :], lhsT=xT[:, :], rhs=W2_sb[:, :],
                                 start=True, stop=True)
                h2sb = xo_p.tile([P, P], F16, tag="h2sb")
                nc.scalar.activation(h2sb[:, :], h2ps[:, :], AF.Copy)
                nc.sync.dma_start(h2_out[b * P:(b + 1) * P, :], h2sb[:, :])
            else:
                x3ps = h2_p.tile([P, P], F16, tag="x3ps")
                nc.tensor.transpose(out=x3ps[:, :], in_=xT[:, :],
                                    identity=ident_sb[:, :])
                x3sb = xo_p.tile([P, P], F16, tag="x3sb")
                nc.scalar.activation(x3sb[:, :], x3ps[:, :], AF.Copy)
                nc.sync.dma_start(x3_tile[b * P:(b + 1) * P, :], x3sb[:, :])


def build_ab(meta):
    NB, K, Vpad = meta["NB"], meta["K"], meta["Vpad"]
    J = NB * K
    nc = bacc.Bacc("TRN2", target_bir_lowering=False, debug=False,
                   num_devices=NCORES)
    embp = nc.dram_tensor("embp", [Vpad, EMB], F16, kind="ExternalInput")
    W1 = nc.dram_tensor("W1", [EMB, HID], F16, kind="ExternalInput")
    W2 = nc.dram_tensor("W2", [HID, HID], F16, kind="ExternalInput")
    b1 = nc.dram_tensor("b1", [HID, 1], F32, kind="ExternalInput")
    iota = nc.dram_tensor("iota", [P, P], F16, kind="ExternalInput")
    idx1 = nc.dram_tensor("idx1", [P, J], I32, kind="ExternalInput")
    dstl = nc.dram_tensor("dstl", [P, J], F32, kind="ExternalInput")
    degs = nc.dram_tensor("degs", [P, J], F16, kind="ExternalInput")
    degd = nc.dram_tensor("degd", [P, J], F16, kind="ExternalInput")
    h2 = nc.dram_tensor("h2", [NB * P, HID], F16, kind="ExternalOutput")

    from contextlib import ExitStack
    with tile.TileContext(nc) as tc, ExitStack() as ctx:
        const_p = ctx.enter_context(tc.tile_pool(name="constp", bufs=1))
        dram_p = ctx.enter_context(tc.tile_pool(name="dramp", bufs=1, space="DRAM"))
        ew_ps = ctx.enter_context(tc.tile_pool(name="ewps", bufs=2, space="PSUM"))

        embT = const_p.tile([EMB, Vpad], F16)
        nc.sync.dma_start_transpose(embT[:, :], embp[:, :])
        W1_sb = const_p.tile([EMB, HID], F16)
        nc.sync.dma_start(W1_sb[:, :], W1[:, :])
        W2_sb = const_p.tile([HID, HID], F16)
        nc.sync.dma_start(W2_sb[:, :], W2[:, :])
        b1_sb = const_p.tile([HID, 1], F32)
        nc.sync.dma_start(b1_sb[:, :], b1[:, :])
        iota_sb = const_p.tile([P, P], F16)
        nc.sync.dma_start(iota_sb[:, :], iota[:, :])

        embW1 = dram_p.tile([Vpad, HID], F16)
        for vb in range(Vpad // P):
            ps = ew_ps.tile([P, HID], F32, tag="ewb")
            nc.tensor.matmul(ps[:, :], lhsT=embT[:, vb * P:(vb + 1) * P],
                             rhs=W1_sb[:, :], start=True, stop=True)
            ew = const_p.tile([P, HID], F16, tag="ewsb")
            nc.scalar.activation(ew[:, :], ps[:, :], AF.Copy)
            nc.sync.dma_start(embW1[vb * P:(vb + 1) * P, :], ew[:, :])

        _edge_layer(nc, tc, ctx, NB, K, embW1[:, :], idx1.ap(), dstl.ap(),
                    degs.ap(), degd.ap(), iota_sb, b1_sb, W2_sb, h2.ap(),
                    None, None)
    nc.compile()
    return nc


def build_c(meta):
    NB, K, GB, K_pool, Lpad = (meta["NB"], meta["K"], meta["GB"],
                               meta["K_pool"], meta["Lpad"])
    J = NB * K
    Jp = GB * K_pool
    TBL = NCORES * Lpad
    nc = bacc.Bacc("TRN2", target_bir_lowering=False, debug=False,
                   num_devices=NCORES)
    h2tab = nc.dram_tensor("h2tab", [TBL, HID], F16, kind="ExternalInput")
    idx2 = nc.dram_tensor("idx2", [P, J], I32, kind="ExternalInput")
    dstl = nc.dram_tensor("dstl", [P, J], F32, kind="ExternalInput")
    degs = nc.dram_tensor("degs", [P, J], F16, kind="ExternalInput")
    degd = nc.dram_tensor("degd", [P, J], F16, kind="ExternalInput")
    b2 = nc.dram_tensor("b2", [HID, 1], F32, kind="ExternalInput")
    iota = nc.dram_tensor("iota", [P, P], F16, kind="ExternalInput")
    ident = nc.dram_tensor("ident", [P, P], F16, kind="ExternalInput")
    Wout = nc.dram_tensor("Wout", [HID, NCLS], F16, kind="ExternalInput")
    bout = nc.dram_tensor("bout", [1, NCLS], F32, kind="ExternalInput")
    poolidx = nc.dram_tensor("poolidx", [P, Jp], I32, kind="ExternalInput")
    batchrel = nc.dram_tensor("batchrel", [P, Jp], F32, kind="ExternalInput")
    out = nc.dram_tensor("out", [GB * P, NCLS], F32, kind="ExternalOutput")

    from contextlib import ExitStack
    with tile.TileContext(nc) as tc, ExitStack() as ctx:
        const_p = ctx.enter_context(tc.tile_pool(name="constp", bufs=1))
        dram_p = ctx.enter_context(tc.tile_pool(name="dramp", bufs=1, space="DRAM"))

        b2_sb = const_p.tile([HID, 1], F32)
        nc.sync.dma_start(b2_sb[:, :], b2[:, :])
        iota_sb = const_p.tile([P, P], F16)
        nc.sync.dma_start(iota_sb[:, :], iota[:, :])
        ident_sb = const_p.tile([P, P], F16)
        nc.sync.dma_start(ident_sb[:, :], ident[:, :])
        Wout_sb = const_p.tile([HID, NCLS], F16)
        nc.sync.dma_start(Wout_sb[:, :], Wout[:, :])
        bout_sb = const_p.tile([1, NCLS], F32)
        nc.sync.dma_start(bout_sb[:, :], bout[:, :])
        bout_bc = const_p.tile([P, NCLS], F32)
        nc.gpsimd.partition_broadcast(bout_bc[:, :], bout_sb[:, :])
        ones_sb = const_p.tile([P, 1], F16)
        nc.vector.memset(ones_sb[:, :], 1.0)

        x3d = dram_p.tile([NB * P, HID], F16)

        _edge_layer(nc, tc, ctx, NB, K, h2tab.ap(), idx2.ap(), dstl.ap(),
                    degs.ap(), degd.ap(), iota_sb, b2_sb, None, None,
                    ident_sb, x3d)

        pool_p = ctx.enter_context(tc.tile_pool(name="poolp", bufs=2))
        pps = ctx.enter_context(tc.tile_pool(name="poolps", bufs=1, space="PSUM"))
        cps = ctx.enter_context(tc.tile_pool(name="cntps", bufs=1, space="PSUM"))
        for g in range(GB):
            pidx_t = pool_p.tile([P, K_pool], I32, tag="pidx")
            nc.sync.dma_start(pidx_t[:, :], poolidx[:, g * K_pool:(g + 1) * K_pool])
            brel_t = pool_p.tile([P, K_pool], F32, tag="brel")
            nc.sync.dma_start(brel_t[:, :], batchrel[:, g * K_pool:(g + 1) * K_pool])
            x3p_t = pool_p.tile([P, K_pool * P], F16, tag="x3p")
            for k in range(K_pool):
                nc.gpsimd.indirect_dma_start(
                    out=x3p_t[:, k * P:(k + 1) * P], out_offset=None, in_=x3d[:, :],
                    in_offset=IndirectOffsetOnAxis(ap=pidx_t[:, k:k + 1], axis=0))
            poolps = pps.tile([P, P], F32, tag="poolps")
            cntps = cps.tile([P, 1], F32, tag="cntps")
            for k in range(K_pool):
                mp = pool_p.tile([P, P], F16, tag="mp")
                nc.vector.tensor_scalar(
                    out=mp[:, :], in0=iota_sb[:, :],
                    scalar1=brel_t[:, k:k + 1], scalar2=None, op0=OP.is_equal)
                nc.tensor.matmul(poolps[:, :], lhsT=x3p_t[:, k * P:(k + 1) * P],
                                 rhs=mp[:, :], start=(k == 0), stop=(k == K_pool - 1))
                nc.tensor.matmul(cntps[:, :], lhsT=mp[:, :], rhs=ones_sb[:, :],
                                 start=(k == 0), stop=(k == K_pool - 1))
            cntm = pool_p.tile([P, 1], F32, tag="cntm")
            nc.vector.tensor_scalar_max(cntm[:, :], cntps[:, :], 1.0)
            rec = pool_p.tile([P, 1], F32, tag="rec")
            nc.vector.reciprocal(rec[:, :], cntm[:, :])
            poolT = pool_p.tile([P, P], F16, tag="poolT")
            nc.scalar.activation(poolT[:, :], poolps[:, :], AF.Copy)
            headps = cps.tile([P, NCLS], F32, tag="headps")
            nc.tensor.matmul(headps[:, :], lhsT=poolT[:, :], rhs=Wout_sb[:, :],
                             start=True, stop=True)
            osb = pool_p.tile([P, NCLS], F32, tag="osb")
            nc.vector.tensor_scalar(out=osb[:, :], in0=headps[:, :],
                                    scalar1=rec[:, :], scalar2=None, op0=OP.mult)
            osb2 = pool_p.tile([P, NCLS], F32, tag="osb2")
            nc.vector.tensor_tensor(out=osb2[:, :], in0=osb[:, :],
                                    in1=bout_bc[:, :], op=OP.add)
            nc.sync.dma_start(out[g * P:(g + 1) * P, :], osb2[:, :])
    nc.compile()
    return nc


# ---------------------------------------------------------------- entry point


_CACHE = {}
LAST_TIMES = {}


def _shared_inputs(inputs, meta):
    Vpad = meta["Vpad"]
    V = inputs["embed"].shape[0]
    embp = np.zeros((Vpad, EMB), np.float16)
    embp[:V] = inputs["embed"].astype(np.float16)
    iota = np.tile(np.arange(P, dtype=np.float16), (P, 1))
    ident = np.eye(P, dtype=np.float16)
    return dict(
        embp=embp,
        W1=np.asarray(inputs["W1"], np.float16),
        W2=np.asarray(inputs["W2"], np.float16),
        Wout=np.asarray(inputs["Wout"], np.float16),
        b1=np.asarray(inputs["b1"], np.float32).reshape(HID, 1),
        b2=np.asarray(inputs["b2"], np.float32).reshape(HID, 1),
        bout=np.asarray(inputs["bout"], np.float32).reshape(1, NCLS),
        iota=iota, ident=ident)


def kernel(node_ids, edge_index, batch, embed, W1, b1, W2, b2, Wout, bout,
           n_graphs=8192):
    from concourse import bass_utils
    inputs = dict(embed=embed, W1=W1, b1=b1, W2=W2, b2=b2, Wout=Wout, bout=bout)
    cores, meta = _prep(node_ids, edge_index, batch, n_graphs, embed.shape[0])
    sh = _shared_inputs(inputs, meta)

    key = ("ab", meta["NB"], meta["K"], meta["Vpad"])
    if key not in _CACHE:
        _CACHE[key] = build_ab(meta)
    nc_ab = _CACHE[key]
    in_ab = [dict(embp=sh["embp"], W1=sh["W1"], W2=sh["W2"], b1=sh["b1"],
                  iota=sh["iota"], idx1=c["idx1"], dstl=c["dstl"],
                  degs=c["degs"], degd=c["degd"]) for c in cores]
    res_ab = bass_utils.run_bass_kernel_spmd(nc_ab, in_ab, list(range(NCORES)))
    LAST_TIMES["ab"] = res_ab.exec_time_ns
    h2tab = np.concatenate([res_ab.results[c]["h2"] for c in range(NCORES)], 0)
    h2tab = np.ascontiguousarray(h2tab.astype(np.float16))

    key2 = ("c", meta["NB"], meta["K"], meta["GB"], meta["K_pool"])
    if key2 not in _CACHE:
        _CACHE[key2] = build_c(meta)
    nc_c = _CACHE[key2]
    in_c = [dict(h2tab=h2tab, idx2=c["idx2"], dstl=c["dstl"], degs=c["degs"],
                 degd=c["degd"], b2=sh["b2"], iota=sh["iota"], ident=sh["ident"],
                 Wout=sh["Wout"], bout=sh["bout"], poolidx=c["poolidx"],
                 batchrel=c["batchrel"]) for c in cores]
    res_c = bass_utils.run_bass_kernel_spmd(nc_c, in_c, list(range(NCORES)))
    LAST_TIMES["c"] = res_c.exec_time_ns
    Gpc = meta["Gpc"]
    out = np.concatenate(
        [res_c.results[c]["out"][:Gpc] for c in range(NCORES)], 0)
    return out.astype(np.float32)



# revision 2
# speedup vs baseline: 1.0294x; 1.0294x over previous
"""Trainium2 Bass kernel for a 2-layer GCN graph classifier — v2.

Key changes vs baseline:
  - All per-edge row gathers use ONE batched SWDGE dma_gather per
    superblock (994ns fixed + 0.34ns/descriptor) instead of one
    indirect_dma_start per 128 edges (994ns each).
  - Layer-2 gathers from a per-core ROTATED h2 table, split in 4
    quarters so local indices fit dma_gather's int16 limit; the
    rotation puts each core's own slice (and thus all self-loops)
    in quarter 0 so chunk counts stay uniform across the SPMD cores.
  - h2 rows are pre-scaled by dinv on write; layer-2 aggregation is
    node-major (lhsT = one-hot) with dinv_dst folded into the relu's
    per-partition scale and the bias added via a rank-1 matmul.
  - Per-core descending block sort so the shared instruction stream's
    per-slot chunk counts (max over cores) pad tightly.
  - Host precomputes embed@W1, all norms/indices as fp16/int16.
"""

import sys

sys.path.insert(0, "/opt/trn_rl_repo")

import numpy as np

import concourse.bacc as bacc
import concourse.bass as bass
import concourse.mybir as mybir
import concourse.tile as tile

P = 128
NCORES = 8
F16 = mybir.dt.float16
F32 = mybir.dt.float32
I16 = mybir.dt.int16
AF = mybir.ActivationFunctionType
OP = mybir.AluOpType

EMB = 64
HID = 128
NCLS = 16
CHUNK_SB = 96  # target chunks per superblock (msg tile = CHUNK_SB*256B/partition)


def _ceil(a, b):
    return -(-a // b)


def _wrap_idx(vals):
    """Positions -> dma_gather wrapped layout [128, N/16] int16."""
    n = vals.shape[0]
    assert n % 16 == 0
    w = vals.reshape(n // 16, 16).T.astype(np.int16)  # [16, n/16]
    return np.ascontiguousarray(np.tile(w, (8, 1)))


def _superblocks(kslot):
    """Group block slots into superblocks of <= CHUNK_SB chunks."""
    sbs = []
    cur = []
    tot = 0
    for b, k in enumerate(kslot):
        if cur and tot + k > CHUNK_SB:
            sbs.append(cur)
            cur = []
            tot = 0
        cur.append(b)
        tot += k
    if cur:
        sbs.append(cur)
    return sbs


# ---------------------------------------------------------------- host prep


def _prep(node_ids, edge_index, batch, n_graphs):
    N = node_ids.shape[0]
    src = np.asarray(edge_index[0], np.int64)
    dst = np.asarray(edge_index[1], np.int64)
    batch = np.asarray(batch, np.int64)
    node_ids = np.asarray(node_ids, np.int64)

    Gpc = n_graphs // NCORES
    cuts = np.searchsorted(batch, np.arange(NCORES + 1) * Gpc)
    deg = (np.bincount(dst, minlength=N) + 1).astype(np.float64)
    dinv = 1.0 / np.sqrt(deg)
    core_of = np.searchsorted(cuts[1:], np.arange(N), side="right")

    dstcore = np.searchsorted(cuts[1:], dst, side="right")

    # ---- per-core edge lists (incl. self-loops), dst-local block ids
    edges = []  # per core: (es, ed) with ed local
    for c in range(NCORES):
        m = dstcore == c
        es = np.concatenate([src[m], np.arange(cuts[c], cuts[c + 1])])
        ed = np.concatenate([dst[m], np.arange(cuts[c], cuts[c + 1])])
        edges.append((es, ed - cuts[c]))

    Ls = cuts[1:] - cuts[:-1]
    nb_core = [int(_ceil(int(l), P)) for l in Ls]
    NB = max(nb_core)
    Lpad = NB * P

    # ================= Layer 1 block assignment ==============
    # per core: per-block edge counts -> sort desc -> slot chunk counts
    cnt1 = np.zeros((NCORES, NB), np.int64)
    for c in range(NCORES):
        es, edl = edges[c]
        np.add.at(cnt1[c], edl >> 7, 1)
    order1 = np.argsort(-cnt1, axis=1, kind="stable")  # slot -> block
    k1 = np.take_along_axis(_ceil(cnt1, P), order1, axis=1)
    K1_slot = k1.max(axis=0)  # [NB]
    NB1 = int(np.max(np.nonzero(K1_slot)[0])) + 1 if K1_slot.any() else 0
    K1_slot = K1_slot[:NB1]
    off1 = np.zeros(NB1 + 1, np.int64)
    off1[1:] = np.cumsum(K1_slot)
    J1 = int(off1[-1])

    # node -> layer1 slot row (within core): block slot of its block * P + pos
    slot1 = []  # per core: local node idx -> row in h2 output
    inv_order1 = np.empty_like(order1)
    for c in range(NCORES):
        inv_order1[c, order1[c]] = np.arange(NB)
    for c in range(NCORES):
        loc = np.arange(Ls[c])
        slot1.append(inv_order1[c, loc >> 7] * P + (loc & 127))

    # ================= Layer 2 (quarters) =====================
    # local table row (rotated): ((core_of(s) - c) % 8) * Lpad + slot1[s]
    # quarter = rot_slice // 2
    K2q = np.zeros((NCORES, NB, 4), np.int64)
    e2meta = []
    for c in range(NCORES):
        es, edl = edges[c]
        rot = (core_of[es] - c) % NCORES
        s_loc = es - cuts[core_of[es]]
        sl = np.empty(len(es), np.int64)
        for cc in range(NCORES):
            m = core_of[es] == cc
            if m.any():
                sl[m] = slot1[cc][s_loc[m]]
        lidx = rot * Lpad + sl
        quarter = rot >> 1
        qidx = lidx - quarter * 2 * Lpad
        blk = edl >> 7
        np.add.at(K2q[c], (blk, quarter), 1)
        e2meta.append((qidx, quarter, blk, edl))
    cnt2 = K2q.sum(axis=2)
    order2 = np.argsort(-cnt2, axis=1, kind="stable")
    k2q_sorted = np.take_along_axis(
        _ceil(K2q, P), order2[:, :, None], axis=1
    )  # [C, NB, 4]
    K2q_slot = k2q_sorted.max(axis=0)  # [NB, 4]
    nz2 = np.nonzero(K2q_slot.sum(axis=1))[0]
    NB2 = int(nz2.max()) + 1 if len(nz2) else 0
    K2q_slot = K2q_slot[:NB2]
    inv_order2 = np.empty_like(order2)
    for c in range(NCORES):
        inv_order2[c, order2[c]] = np.arange(NB)
    slot2 = []
    for c in range(NCORES):
        loc = np.arange(Ls[c])
        slot2.append(inv_order2[c, loc >> 7] * P + (loc & 127))

    # superblocks over layer2 slots by total chunks
    K2_tot = K2q_slot.sum(axis=1)
    sbs2 = _superblocks(K2_tot)
    # chunk column layout: per SB, quarter-major runs
    # col2[b][q] = start column (global) of block b's quarter-q run
    col2 = np.zeros((NB2, 4), np.int64)
    sb2_info = []  # per SB: (blocks, Js, col0, qruns=[(q, c0, c1)])
    colg = 0
    for blocks in sbs2:
        col0 = colg
        qruns = []
        for q in range(4):
            c0 = colg
            for b in blocks:
                col2[b, q] = colg
                colg += int(K2q_slot[b, q])
            qruns.append((q, c0, colg))
        sb2_info.append((blocks, colg - col0, col0, qruns))
    J2 = colg

    # superblocks layer 1
    sbs1 = _superblocks(K1_slot)
    sb1_info = []
    for blocks in sbs1:
        col0 = int(off1[blocks[0]])
        Js = int(off1[blocks[-1] + 1] - col0)
        sb1_info.append((blocks, Js, col0))

    # ================= Pool =====================
    GB = _ceil(Gpc, P)
    cntp = np.zeros((NCORES, GB), np.int64)
    for c in range(NCORES):
        gl = batch[cuts[c]:cuts[c + 1]] - c * Gpc
        np.add.at(cntp[c], gl >> 7, 1)
    orderp = np.argsort(-cntp, axis=1, kind="stable")
    kp = np.take_along_axis(_ceil(cntp, P), orderp, axis=1)
    Kp_slot = kp.max(axis=0)
    Kp_slot = np.maximum(Kp_slot, 1)
    offp = np.zeros(GB + 1, np.int64)
    offp[1:] = np.cumsum(Kp_slot)
    Jp = int(offp[-1])
    inv_orderp = np.empty_like(orderp)
    for c in range(NCORES):
        inv_orderp[c, orderp[c]] = np.arange(GB)

    # ================= per-core data fill =====================
    cores = []
    for c in range(NCORES):
        es, edl = edges[c]
        qidx, quarter, blk, _ = e2meta[c]

        # ---- L1 fill
        idx1 = np.zeros(J1 * P, np.int64)
        dstl1 = np.full((P, J1), -1.0, np.float32)
        norm1 = np.zeros((P, J1), np.float32)
        bslot1 = inv_order1[c, blk]
        # position within block-slot's chunk run
        o = np.argsort(bslot1, kind="stable")
        es_o, edl_o, bs_o = es[o], edl[o], bslot1[o]
        start = np.zeros(NB1 + 1, np.int64)
        np.add.at(start, bs_o + 1, 1)
        start = np.cumsum(start)
        rank = np.arange(len(es_o)) - start[bs_o]
        pos = (off1[bs_o] + (rank >> 7)) * P + (rank & 127)
        idx1[pos] = node_ids[es_o]
        dstl1[pos & 127, pos >> 7] = (edl_o & 127).astype(np.float32)
        nrm = dinv[es_o] * dinv[edl_o + cuts[c]]
        norm1[pos & 127, pos >> 7] = nrm.astype(np.float32)

        # ---- L2 fill (quarter-major runs within superblock)
        idx2 = np.zeros(J2 * P, np.int64)
        dstl2 = np.full((P, J2), -1.0, np.float32)
        bslot2 = inv_order2[c, blk]
        key = bslot2 * 4 + quarter
        o = np.argsort(key, kind="stable")
        qx_o, q_o, edl2_o, bs2_o = qidx[o], quarter[o], edl[o], bslot2[o]
        startq = np.zeros(NB2 * 4 + 1, np.int64)
        np.add.at(startq, bs2_o * 4 + q_o + 1, 1)
        startq = np.cumsum(startq)
        rank = np.arange(len(qx_o)) - startq[bs2_o * 4 + q_o]
        pos = (col2[bs2_o, q_o] + (rank >> 7)) * P + (rank & 127)
        idx2[pos] = qx_o
        dstl2[pos & 127, pos >> 7] = (edl2_o & 127).astype(np.float32)

        # ---- per-node columns (dinv etc.), L1 & L2 slot layouts
        dinv1col = np.ones((P, NB1), np.float32)
        dinv2col = np.ones((P, NB2), np.float32)
        sqdeg = np.zeros(NB2 * P, np.float16)
        loc = np.arange(Ls[c])
        dv = dinv[cuts[c]:cuts[c + 1]]
        r1 = slot1[c]
        dinv1col[r1 & 127, r1 >> 7] = dv.astype(np.float32)
        r2 = slot2[c]
        dinv2col[r2 & 127, r2 >> 7] = dv.astype(np.float32)
        sqdeg[r2] = np.sqrt(deg[cuts[c]:cuts[c + 1]]).astype(np.float16)

        # ---- pool fill
        gl = batch[cuts[c]:cuts[c + 1]] - c * Gpc
        gslot = inv_orderp[c, gl >> 7]
        o = np.argsort(gslot, kind="stable")
        loc_o, gl_o, gs_o = loc[o], gl[o], gslot[o]
        startp = np.zeros(GB + 1, np.int64)
        np.add.at(startp, gs_o + 1, 1)
        startp = np.cumsum(startp)
        rank = np.arange(len(loc_o)) - startp[gs_o]
        pos = (offp[gs_o] + (rank >> 7)) * P + (rank & 127)
        pidx = np.zeros(Jp * P, np.int64)
        grel = np.full((P, Jp), -1.0, np.float32)
        pidx[pos] = slot2[c][loc_o]
        grel[pos & 127, pos >> 7] = (gl_o & 127).astype(np.float32)

        cores.append(dict(
            idx1w=_wrap_idx(idx1), dstl1=dstl1, norm1=norm1,
            idx2w=_wrap_idx(idx2), dstl2=dstl2,
            dinv1col=dinv1col, dinv2col=dinv2col,
            sqdeg=sqdeg.reshape(1, NB2 * P),
            pidxw=_wrap_idx(pidx), grel=grel,
        ))

    meta = dict(NB1=NB1, NB2=NB2, J1=J1, J2=J2, Jp=Jp, GB=GB, Gpc=Gpc,
                Lpad=Lpad,
                K1_slot=tuple(int(x) for x in K1_slot),
                K2q_slot=tuple(tuple(int(x) for x in r) for r in K2q_slot),
                Kp_slot=tuple(int(x) for x in Kp_slot),
                sb1_info=tuple((tuple(b), js, c0) for b, js, c0 in sb1_info),
                sb2_info=tuple(
                    (tuple(b), js, c0, tuple(qr)) for b, js, c0, qr in sb2_info),
                off1=tuple(int(x) for x in off1),
                col2=tuple(tuple(int(x) for x in r) for r in col2),
                offp=tuple(int(x) for x in offp))
    aux = dict(slot1=slot1, slot2=slot2, inv_orderp=inv_orderp, cuts=cuts,
               Ls=Ls)
    return cores, meta, aux


# ------------------------------------------------------------ launch AB


def build_ab(meta, vpad, has_b2):
    NB1, J1, Lpad = meta["NB1"], meta["J1"], meta["Lpad"]
    K1_slot, sb1_info, off1 = meta["K1_slot"], meta["sb1_info"], meta["off1"]
    nc = bacc.Bacc("TRN2", target_bir_lowering=False, debug=False,
                   num_devices=NCORES)
    embW1 = nc.dram_tensor("embW1", [vpad, HID], F16, kind="ExternalInput")
    idx1w = nc.dram_tensor("idx1w", [P, J1 * 8], I16, kind="ExternalInput")
    dstl1 = nc.dram_tensor("dstl1", [P, J1], F32, kind="ExternalInput")
    norm1 = nc.dram_tensor("norm1", [P, J1], F32, kind="ExternalInput")
    W2 = nc.dram_tensor("W2", [HID, HID], F16, kind="ExternalInput")
    b1 = nc.dram_tensor("b1", [HID, 1], F32, kind="ExternalInput")
    dinv1col = nc.dram_tensor("dinv1col", [P, NB1], F32, kind="ExternalInput")
    iota = nc.dram_tensor("iota", [P, P], F16, kind="ExternalInput")
    h2 = nc.dram_tensor("h2", [Lpad, HID], F16, kind="ExternalOutput")

    from contextlib import ExitStack
    with tile.TileContext(nc) as tc, ExitStack() as ctx:
        const_p = ctx.enter_context(tc.tile_pool(name="constp", bufs=1))
        W2_sb = const_p.tile([HID, HID], F16)
        nc.sync.dma_start(W2_sb[:, :], W2[:, :])
        b1_sb = const_p.tile([HID, 1], F32)
        nc.sync.dma_start(b1_sb[:, :], b1[:, :])
        iota_sb = const_p.tile([P, P], F16)
        nc.sync.dma_start(iota_sb[:, :], iota[:, :])
        dinv_sb = const_p.tile([P, NB1], F32)
        nc.sync.dma_start(dinv_sb[:, :], dinv1col[:, :])

        idx_p = ctx.enter_context(tc.tile_pool(name="idxp", bufs=2))
        msg_p = ctx.enter_context(tc.tile_pool(name="msgp", bufs=2))
        mt_p = ctx.enter_context(tc.tile_pool(name="mtp", bufs=4))
        xo_p = ctx.enter_context(tc.tile_pool(name="xop", bufs=3))
        agg_p = ctx.enter_context(tc.tile_pool(name="aggps", bufs=2, space="PSUM"))
        h2_p = ctx.enter_context(tc.tile_pool(name="h2ps", bufs=2, space="PSUM"))

        for blocks, Js, col0 in sb1_info:
            idx_t = idx_p.tile([P, Js * 8], I16, tag="idx")
            nc.sync.dma_start(idx_t[:, :], idx1w[:, col0 * 8:(col0 + Js) * 8])
            dstl_t = idx_p.tile([P, Js], F32, tag="dstl")
            nc.sync.dma_start(dstl_t[:, :], dstl1[:, col0:col0 + Js])
            norm_t = idx_p.tile([P, Js], F32, tag="norm")
            nc.sync.dma_start(norm_t[:, :], norm1[:, col0:col0 + Js])

            msg_t = msg_p.tile([P, Js, HID], F16, tag="msg")
            nc.gpsimd.dma_gather(
                msg_t[:, :, :], embW1[:, :], idx_t[:, :],
                num_idxs=Js * P, num_idxs_reg=Js * P, elem_size=HID, single_packet=False)

            for b in blocks:
                K = K1_slot[b]
                agg = agg_p.tile([P, P], F32, tag="agg")
                for k in range(K):
                    j = off1[b] - col0 + k
                    mt = mt_p.tile([P, P], F16, tag="mt")
                    nc.vector.tensor_scalar(
                        out=mt[:, :], in0=iota_sb[:, :],
                        scalar1=dstl_t[:, j:j + 1], scalar2=norm_t[:, j:j + 1],
                        op0=OP.is_equal, op1=OP.mult)
                    nc.tensor.matmul(agg[:, :], lhsT=msg_t[:, j, :],
                                     rhs=mt[:, :], start=(k == 0),
                                     stop=(k == K - 1))
                # agg is [h, d]; relu + bias(per-partition h)
                xT = xo_p.tile([P, P], F16, tag="xT")
                nc.scalar.activation(xT[:, :], agg[:, :], AF.Relu,
                                     bias=b1_sb[:, :])
                h2ps = h2_p.tile([P, P], F32, tag="h2ps")
                nc.tensor.matmul(h2ps[:, :], lhsT=xT[:, :], rhs=W2_sb[:, :],
                                 start=True, stop=True)
                h2sb = xo_p.tile([P, P], F16, tag="h2sb")
                nc.scalar.activation(h2sb[:, :], h2ps[:, :], AF.Copy,
                                     scale=dinv_sb[:, b:b + 1])
                nc.sync.dma_start(h2[b * P:(b + 1) * P, :], h2sb[:, :])
    nc.compile()
    return nc


# ------------------------------------------------------------ launch C


def build_c(meta, has_b2):
    NB2, J2, Jp, GB, Lpad = (meta["NB2"], meta["J2"], meta["Jp"], meta["GB"],
                             meta["Lpad"])
    K2q_slot, sb2_info, col2 = meta["K2q_slot"], meta["sb2_info"], meta["col2"]
    Kp_slot, offp = meta["Kp_slot"], meta["offp"]
    TBL = NCORES * Lpad
    QROWS = 2 * Lpad
    nc = bacc.Bacc("TRN2", target_bir_lowering=False, debug=False,
                   num_devices=NCORES)
    h2tab = nc.dram_tensor("h2tab", [TBL, HID], F16, kind="ExternalInput")
    idx2w = nc.dram_tensor("idx2w", [P, J2 * 8], I16, kind="ExternalInput")
    dstl2 = nc.dram_tensor("dstl2", [P, J2], F32, kind="ExternalInput")
    sqdeg = nc.dram_tensor("sqdeg", [1, NB2 * P], F16, kind="ExternalInput")
    b2row = nc.dram_tensor("b2row", [1, HID], F16, kind="ExternalInput")
    dinv2col = nc.dram_tensor("dinv2col", [P, NB2], F32, kind="ExternalInput")
    iota = nc.dram_tensor("iota", [P, P], F16, kind="ExternalInput")
    pidxw = nc.dram_tensor("pidxw", [P, Jp * 8], I16, kind="ExternalInput")
    grel = nc.dram_tensor("grel", [P, Jp], F32, kind="ExternalInput")
    Wout = nc.dram_tensor("Wout", [HID, NCLS], F16, kind="ExternalInput")
    bout = nc.dram_tensor("bout", [1, NCLS], F32, kind="ExternalInput")
    out = nc.dram_tensor("out", [GB * P, NCLS], F32, kind="ExternalOutput")

    from contextlib import ExitStack
    with tile.TileContext(nc) as tc, ExitStack() as ctx:
        const_p = ctx.enter_context(tc.tile_pool(name="constp", bufs=1))
        dram_p = ctx.enter_context(tc.tile_pool(name="dramp", bufs=1,
                                                space="DRAM"))
        iota_sb = const_p.tile([P, P], F16)
        nc.sync.dma_start(iota_sb[:, :], iota[:, :])
        dinv_sb = const_p.tile([P, NB2], F32)
        nc.sync.dma_start(dinv_sb[:, :], dinv2col[:, :])
        Wout_sb = const_p.tile([HID, NCLS], F16)
        nc.sync.dma_start(Wout_sb[:, :], Wout[:, :])
        bout_sb = const_p.tile([1, NCLS], F32)
        nc.sync.dma_start(bout_sb[:, :], bout[:, :])
        bout_bc = const_p.tile([P, NCLS], F32)
        nc.gpsimd.partition_broadcast(bout_bc[:, :], bout_sb[:, :])
        ones_sb = const_p.tile([P, 1], F16)
        nc.vector.memset(ones_sb[:, :], 1.0)
        if has_b2:
            sq_sb = const_p.tile([1, NB2 * P], F16)
            nc.sync.dma_start(sq_sb[:, :], sqdeg[:, :])
            b2_sb = const_p.tile([1, HID], F16)
            nc.sync.dma_start(b2_sb[:, :], b2row[:, :])

        x3d = dram_p.tile([Lpad, HID], F16)

        idx_p = ctx.enter_context(tc.tile_pool(name="idxp", bufs=2))
        msg_p = ctx.enter_context(tc.tile_pool(name="msgp", bufs=2))
        mt_p = ctx.enter_context(tc.tile_pool(name="mtp", bufs=4))
        xo_p = ctx.enter_context(tc.tile_pool(name="xop", bufs=3))
        agg_p = ctx.enter_context(tc.tile_pool(name="aggps", bufs=2, space="PSUM"))

        for blocks, Js, col0, qruns in sb2_info:
            idx_t = idx_p.tile([P, Js * 8], I16, tag="idx")
            nc.sync.dma_start(idx_t[:, :], idx2w[:, col0 * 8:(col0 + Js) * 8])
            dstl_t = idx_p.tile([P, Js], F32, tag="dstl")
            nc.sync.dma_start(dstl_t[:, :], dstl2[:, col0:col0 + Js])

            msg_t = msg_p.tile([P, Js, HID], F16, tag="msg")
            for q, c0, c1 in qruns:
                if c1 == c0:
                    continue
                nq = c1 - c0
                r0 = c0 - col0
                nc.gpsimd.dma_gather(
                    msg_t[:, r0:r0 + nq, :],
                    h2tab[q * QROWS:(q + 1) * QROWS, :],
                    idx_t[:, r0 * 8:(r0 + nq) * 8],
                    num_idxs=nq * P, num_idxs_reg=nq * P, elem_size=HID, single_packet=False)

            for b in blocks:
                agg = agg_p.tile([P, P], F32, tag="agg")
                first = True
                nchunks = sum(K2q_slot[b])
                done = 0
                if has_b2:
                    nc.tensor.matmul(
                        agg[:, :], lhsT=sq_sb[:, b * P:(b + 1) * P],
                        rhs=b2_sb[:, :], start=True, stop=False)
                    first = False
                for q in range(4):
                    for k in range(K2q_slot[b][q]):
                        j = col2[b][q] - col0 + k
                        done += 1
                        mt = mt_p.tile([P, P], F16, tag="mt")
                        nc.vector.tensor_scalar(
                            out=mt[:, :], in0=iota_sb[:, :],
                            scalar1=dstl_t[:, j:j + 1], scalar2=None,
                            op0=OP.is_equal)
                        nc.tensor.matmul(agg[:, :], lhsT=mt[:, :],
                                         rhs=msg_t[:, j, :], start=first,
                                         stop=(done == nchunks))
                        first = False
                # agg is [d, h]; x3 = relu(dinv_d * agg (+ b2))
                x3sb = xo_p.tile([P, P], F16, tag="x3sb")
                nc.scalar.activation(x3sb[:, :], agg[:, :], AF.Relu,
                                     scale=dinv_sb[:, b:b + 1])
                nc.sync.dma_start(x3d[b * P:(b + 1) * P, :], x3sb[:, :])

        # ---------------- pool + head
        pool_p = ctx.enter_context(tc.tile_pool(name="poolp", bufs=2))
        pps = ctx.enter_context(tc.tile_pool(name="poolps", bufs=2, space="PSUM"))
        cps = ctx.enter_context(tc.tile_pool(name="cntps", bufs=2, space="PSUM"))
        for g in range(GB):
            Kp = Kp_slot[g]
            c0 = offp[g]
            pidx_t = pool_p.tile([P, Kp * 8], I16, tag="pidx")
            nc.sync.dma_start(pidx_t[:, :], pidxw[:, c0 * 8:(c0 + Kp) * 8])
            grel_t = pool_p.tile([P, Kp], F32, tag="grel")
            nc.sync.dma_start(grel_t[:, :], grel[:, c0:c0 + Kp])
            x3p_t = pool_p.tile([P, Kp, HID], F16, tag="x3p")
            nc.gpsimd.dma_gather(
                x3p_t[:, :, :], x3d[:, :], pidx_t[:, :],
                num_idxs=Kp * P, num_idxs_reg=Kp * P, elem_size=HID, single_packet=False)
            poolps = pps.tile([P, P], F32, tag="poolps")
            cntps = cps.tile([P, 1], F32, tag="cntps")
            for k in range(Kp):
                mp = pool_p.tile([P, P], F16, tag="mp")
                nc.vector.tensor_scalar(
                    out=mp[:, :], in0=iota_sb[:, :],
                    scalar1=grel_t[:, k:k + 1], scalar2=None, op0=OP.is_equal)
                nc.tensor.matmul(poolps[:, :], lhsT=x3p_t[:, k, :],
                                 rhs=mp[:, :], start=(k == 0),
                                 stop=(k == Kp - 1))
                nc.tensor.matmul(cntps[:, :], lhsT=mp[:, :], rhs=ones_sb[:, :],
                                 start=(k == 0), stop=(k == Kp - 1))
            cntm = pool_p.tile([P, 1], F32, tag="cntm")
            nc.vector.tensor_scalar_max(cntm[:, :], cntps[:, :], 1.0)
            rec = pool_p.tile([P, 1], F32, tag="rec")
            nc.vector.reciprocal(rec[:, :], cntm[:, :])
            poolT = pool_p.tile([P, P], F16, tag="poolT")
            nc.scalar.activation(poolT[:, :], poolps[:, :], AF.Copy)
            headps = cps.tile([P, NCLS], F32, tag="headps")
            nc.tensor.matmul(headps[:, :], lhsT=poolT[:, :], rhs=Wout_sb[:, :],
                             start=True, stop=True)
            osb = pool_p.tile([P, NCLS], F32, tag="osb")
            nc.vector.tensor_scalar(out=osb[:, :], in0=headps[:, :],
                                    scalar1=rec[:, :], scalar2=None,
                                    op0=OP.mult)
            osb2 = pool_p.tile([P, NCLS], F32, tag="osb2")
            nc.vector.tensor_tensor(out=osb2[:, :], in0=osb[:, :],
                                    in1=bout_bc[:, :], op=OP.add)
            nc.sync.dma_start(out[g * P:(g + 1) * P, :], osb2[:, :])
    nc.compile()
    return nc


# ---------------------------------------------------------------- entry point


_CACHE = {}
LAST_TIMES = {}


def kernel(node_ids, edge_index, batch, embed, W1, b1, W2, b2, Wout, bout,
           n_graphs=8192):
    from concourse import bass_utils
    cores, meta, aux = _prep(node_ids, edge_index, batch, n_graphs)
    NB1, NB2, GB, Gpc, Lpad = (meta["NB1"], meta["NB2"], meta["GB"],
                               meta["Gpc"], meta["Lpad"])

    V = embed.shape[0]
    vpad = _ceil(V, P) * P
    embW1 = np.zeros((vpad, HID), np.float16)
    embW1[:V] = (np.asarray(embed, np.float64)
                 @ np.asarray(W1, np.float64)).astype(np.float16)
    iota = np.tile(np.arange(P, dtype=np.float16), (P, 1))
    has_b2 = bool(np.any(np.asarray(b2) != 0))

    key = ("ab2", vpad, has_b2) + tuple(
        meta[k] for k in ("NB1", "J1", "K1_slot", "sb1_info", "off1", "Lpad"))
    if key not in _CACHE:
        _CACHE[key] = build_ab(meta, vpad, has_b2)
    nc_ab = _CACHE[key]
    sh = dict(embW1=embW1, W2=np.asarray(W2, np.float16),
              b1=np.asarray(b1, np.float32).reshape(HID, 1), iota=iota)
    in_ab = [dict(sh, idx1w=c["idx1w"], dstl1=c["dstl1"], norm1=c["norm1"],
                  dinv1col=c["dinv1col"]) for c in cores]
    res_ab = bass_utils.run_bass_kernel_spmd(nc_ab, in_ab, list(range(NCORES)))
    LAST_TIMES["ab"] = res_ab.exec_time_ns

    h2all = np.stack([np.asarray(res_ab.results[c]["h2"], np.float16)
                      for c in range(NCORES)])  # [8, Lpad, HID]
    key2 = ("c2", has_b2) + tuple(
        meta[k] for k in ("NB2", "J2", "Jp", "GB", "K2q_slot", "sb2_info",
                          "col2", "Kp_slot", "offp", "Lpad"))
    if key2 not in _CACHE:
        _CACHE[key2] = build_c(meta, has_b2)
    nc_c = _CACHE[key2]
    shc = dict(iota=iota, Wout=np.asarray(Wout, np.float16),
               bout=np.asarray(bout, np.float32).reshape(1, NCLS),
               b2row=np.asarray(b2, np.float16).reshape(1, HID))
    in_c = []
    for c in range(NCORES):
        rot = np.roll(np.arange(NCORES), -c)
        h2tab = np.ascontiguousarray(
            h2all[rot].reshape(NCORES * Lpad, HID))
        in_c.append(dict(shc, h2tab=h2tab, idx2w=cores[c]["idx2w"],
                         dstl2=cores[c]["dstl2"], sqdeg=cores[c]["sqdeg"],
                         dinv2col=cores[c]["dinv2col"],
                         pidxw=cores[c]["pidxw"], grel=cores[c]["grel"]))
    res_c = bass_utils.run_bass_kernel_spmd(nc_c, in_c, list(range(NCORES)))
    LAST_TIMES["c"] = res_c.exec_time_ns

    out = np.empty((n_graphs, NCLS), np.float32)
    for c in range(NCORES):
        o = np.asarray(res_c.results[c]["out"], np.float32)  # [GB*P, NCLS]
        g = np.arange(Gpc)
        rows = aux["inv_orderp"][c, g >> 7] * P + (g & 127)
        out[c * Gpc:(c + 1) * Gpc] = o[rows]
    return out


# revision 3
# speedup vs baseline: 1.0322x; 1.0028x over previous
"""Trainium2 Bass kernel for a 2-layer GCN graph classifier — v2.

Key changes vs baseline:
  - All per-edge row gathers use ONE batched SWDGE dma_gather per
    superblock (994ns fixed + 0.34ns/descriptor) instead of one
    indirect_dma_start per 128 edges (994ns each).
  - Layer-2 gathers from a per-core ROTATED h2 table, split in 4
    quarters so local indices fit dma_gather's int16 limit; the
    rotation puts each core's own slice (and thus all self-loops)
    in quarter 0 so chunk counts stay uniform across the SPMD cores.
  - h2 rows are pre-scaled by dinv on write; layer-2 aggregation is
    node-major (lhsT = one-hot) with dinv_dst folded into the relu's
    per-partition scale and the bias added via a rank-1 matmul.
  - Per-core descending block sort so the shared instruction stream's
    per-slot chunk counts (max over cores) pad tightly.
  - Host precomputes embed@W1, all norms/indices as fp16/int16.
"""

import sys

sys.path.insert(0, "/opt/trn_rl_repo")

import numpy as np

import concourse.bacc as bacc
import concourse.bass as bass
import concourse.mybir as mybir
import concourse.tile as tile

P = 128
NCORES = 8
F16 = mybir.dt.float16
F32 = mybir.dt.float32
I16 = mybir.dt.int16
I32 = mybir.dt.int32
AF = mybir.ActivationFunctionType
OP = mybir.AluOpType

EMB = 64
HID = 128
NCLS = 16
CHUNK_SB = 96  # target chunks per superblock (msg tile = CHUNK_SB*256B/partition)


def _ceil(a, b):
    return -(-a // b)


def _wrap_idx(vals):
    """Positions -> dma_gather wrapped layout [128, N/16] int16."""
    n = vals.shape[0]
    assert n % 16 == 0
    w = vals.reshape(n // 16, 16).T.astype(np.int16)  # [16, n/16]
    return np.ascontiguousarray(np.tile(w, (8, 1)))


def _superblocks(kslot):
    """Group block slots into superblocks of <= CHUNK_SB chunks."""
    sbs = []
    cur = []
    tot = 0
    for b, k in enumerate(kslot):
        if cur and tot + k > CHUNK_SB:
            sbs.append(cur)
            cur = []
            tot = 0
        cur.append(b)
        tot += k
    if cur:
        sbs.append(cur)
    return sbs


# ---------------------------------------------------------------- host prep


def _prep(node_ids, edge_index, batch, n_graphs):
    N = node_ids.shape[0]
    src = np.asarray(edge_index[0], np.int64)
    dst = np.asarray(edge_index[1], np.int64)
    batch = np.asarray(batch, np.int64)
    node_ids = np.asarray(node_ids, np.int64)

    Gpc = n_graphs // NCORES
    cuts = np.searchsorted(batch, np.arange(NCORES + 1) * Gpc)
    deg = (np.bincount(dst, minlength=N) + 1).astype(np.float64)
    dinv = 1.0 / np.sqrt(deg)
    core_of = np.searchsorted(cuts[1:], np.arange(N), side="right")

    dstcore = np.searchsorted(cuts[1:], dst, side="right")

    # ---- per-core edge lists (incl. self-loops), dst-local block ids
    edges = []  # per core: (es, ed) with ed local
    for c in range(NCORES):
        m = dstcore == c
        es = np.concatenate([src[m], np.arange(cuts[c], cuts[c + 1])])
        ed = np.concatenate([dst[m], np.arange(cuts[c], cuts[c + 1])])
        edges.append((es, ed - cuts[c]))

    Ls = cuts[1:] - cuts[:-1]
    nb_core = [int(_ceil(int(l), P)) for l in Ls]
    NB = max(nb_core)
    Lpad = NB * P

    # ============ shared block permutation (sort by total edge count) ====
    cnt1 = np.zeros((NCORES, NB), np.int64)
    for c in range(NCORES):
        es, edl = edges[c]
        np.add.at(cnt1[c], edl >> 7, 1)
    # L2 edge counts exclude self-loops (handled as const identity chunks)
    cnt2 = np.zeros((NCORES, NB), np.int64)
    for c in range(NCORES):
        es, edl = edges[c]
        ns = len(es) - (cuts[c + 1] - cuts[c])
        np.add.at(cnt2[c], edl[:ns] >> 7, 1)
    order = np.argsort(-(cnt1 + cnt2), axis=1, kind="stable")
    inv_order = np.empty_like(order)
    for c in range(NCORES):
        inv_order[c, order[c]] = np.arange(NB)
    slot1 = []
    for c in range(NCORES):
        loc = np.arange(Ls[c])
        slot1.append(inv_order[c, loc >> 7] * P + (loc & 127))

    # ---- L1 chunk slots
    k1 = np.take_along_axis(_ceil(cnt1, P), order, axis=1)
    K1_slot = k1.max(axis=0)
    NB1 = int(np.max(np.nonzero(K1_slot)[0])) + 1 if K1_slot.any() else 0
    K1_slot = np.maximum(K1_slot[:NB1], 1)
    off1 = np.zeros(NB1 + 1, np.int64)
    off1[1:] = np.cumsum(K1_slot)
    J1 = int(off1[-1])
    inv_order1 = inv_order
    order1 = order

    # ================= Layer 2 (quarters, no self-loops) =====================
    K2q = np.zeros((NCORES, NB, 4), np.int64)
    e2meta = []
    for c in range(NCORES):
        es, edl = edges[c]
        ns = len(es) - (cuts[c + 1] - cuts[c])
        es, edl = es[:ns], edl[:ns]
        rot = (core_of[es] - c) % NCORES
        s_loc = es - cuts[core_of[es]]
        sl = np.empty(len(es), np.int64)
        for cc in range(NCORES):
            m = core_of[es] == cc
            if m.any():
                sl[m] = slot1[cc][s_loc[m]]
        lidx = rot * Lpad + sl
        quarter = rot >> 1
        qidx = lidx - quarter * 2 * Lpad
        blk = edl >> 7
        np.add.at(K2q[c], (blk, quarter), 1)
        e2meta.append((qidx, lidx, quarter, blk, edl))
    k2q_sorted = np.take_along_axis(
        _ceil(K2q, P), order[:, :, None], axis=1
    )
    K2q_slot = k2q_sorted.max(axis=0)  # [NB, 4]
    NB2 = NB1  # shared permutation; every block with nodes has a self chunk
    K2q_slot = K2q_slot[:NB2]
    inv_order2 = inv_order
    slot2 = slot1

    # superblocks over layer2 slots by total chunks
    K2_tot = K2q_slot.sum(axis=1)
    sbs2 = _superblocks(K2_tot)
    col2 = np.zeros((NB2, 4), np.int64)
    sb2_info = []
    colg = 0
    for blocks in sbs2:
        col0 = colg
        qruns = []
        for q in range(4):
            c0 = colg
            for b in blocks:
                col2[b, q] = colg
                colg += int(K2q_slot[b, q])
            qruns.append((q, c0, colg))
        sb2_info.append((blocks, colg - col0, col0, qruns))
    J2 = colg

    # superblocks layer 1
    sbs1 = _superblocks(K1_slot)
    sb1_info = []
    for blocks in sbs1:
        col0 = int(off1[blocks[0]])
        Js = int(off1[blocks[-1] + 1] - col0)
        sb1_info.append((blocks, Js, col0))

    # ---- split each superblock's columns between dma_gather (int16 wrapped)
    # and indirect DMA (int32 slot layout).  R_G=7.8ns/row Q7, R_I=20ns/row
    # patcher; C launch gather stream also carries the pool gather.
    FRAC_G1 = 1.0
    sb1_split = []
    for blocks, Js, col0 in sb1_info:
        sb1_split.append(col0 + int(round(Js * FRAC_G1)))
    FRAC_G2 = 1.0
    sb2_split = []
    for blocks, Js, col0, qruns in sb2_info:
        sb2_split.append(col0 + int(round(Js * FRAC_G2)))
    # ================= Pool =====================
    GB = _ceil(Gpc, P)
    cntp = np.zeros((NCORES, GB), np.int64)
    for c in range(NCORES):
        gl = batch[cuts[c]:cuts[c + 1]] - c * Gpc
        np.add.at(cntp[c], gl >> 7, 1)
    orderp = np.argsort(-cntp, axis=1, kind="stable")
    kp = np.take_along_axis(_ceil(cntp, P), orderp, axis=1)
    Kp_slot = kp.max(axis=0)
    Kp_slot = np.maximum(Kp_slot, 1)
    offp = np.zeros(GB + 1, np.int64)
    offp[1:] = np.cumsum(Kp_slot)
    Jp = int(offp[-1])
    inv_orderp = np.empty_like(orderp)
    for c in range(NCORES):
        inv_orderp[c, orderp[c]] = np.arange(GB)

    # ================= per-core data fill =====================
    cores = []
    for c in range(NCORES):
        es, edl = edges[c]

        # ---- L1 fill (includes self-loops)
        idx1 = np.zeros(J1 * P, np.int64)
        dstl1 = np.full((P, J1), -1.0, np.float32)
        norm1 = np.zeros((P, J1), np.float32)
        bslot1 = inv_order1[c, edl >> 7]
        o = np.argsort(bslot1, kind="stable")
        es_o, edl_o, bs_o = es[o], edl[o], bslot1[o]
        start = np.zeros(NB1 + 1, np.int64)
        np.add.at(start, bs_o + 1, 1)
        start = np.cumsum(start)
        rank = np.arange(len(es_o)) - start[bs_o]
        pos = (off1[bs_o] + (rank >> 7)) * P + (rank & 127)
        idx1[pos] = node_ids[es_o]
        dstl1[pos & 127, pos >> 7] = (edl_o & 127).astype(np.float32)
        nrm = dinv[es_o] * dinv[edl_o + cuts[c]]
        norm1[pos & 127, pos >> 7] = nrm.astype(np.float32)

        # ---- L2 fill (quarter-major runs; no self-loops)
        idx2 = np.zeros(J2 * P, np.int64)   # quarter-local (int16 stream)
        idx2g = np.zeros(J2 * P, np.int64)  # global (int32 stream)
        dstl2 = np.full((P, J2), -1.0, np.float32)
        qidx, lidx, quarter, blk, edl2 = e2meta[c]
        bslot2 = inv_order2[c, blk]
        key = bslot2 * 4 + quarter
        o = np.argsort(key, kind="stable")
        qx_o, lx_o, q_o, edl2_o, bs2_o = (qidx[o], lidx[o], quarter[o],
                                          edl2[o], bslot2[o])
        startq = np.zeros(NB2 * 4 + 1, np.int64)
        np.add.at(startq, bs2_o * 4 + q_o + 1, 1)
        startq = np.cumsum(startq)
        rank = np.arange(len(qx_o)) - startq[bs2_o * 4 + q_o]
        pos = (col2[bs2_o, q_o] + (rank >> 7)) * P + (rank & 127)
        idx2[pos] = qx_o
        idx2g[pos] = lx_o
        dstl2[pos & 127, pos >> 7] = (edl2_o & 127).astype(np.float32)

        # ---- per-node columns
        dinv1col = np.ones((P, NB1), np.float32)
        dinv2col = np.ones((P, NB2), np.float32)
        sqdeg = np.zeros(NB2 * P, np.float16)
        loc = np.arange(Ls[c])
        dv = dinv[cuts[c]:cuts[c + 1]]
        r1 = slot1[c]
        dinv1col[r1 & 127, r1 >> 7] = dv.astype(np.float32)
        dinv2col[r1 & 127, r1 >> 7] = dv.astype(np.float32)
        sqdeg[r1] = np.sqrt(deg[cuts[c]:cuts[c + 1]]).astype(np.float16)

        # ---- pool fill
        gl = batch[cuts[c]:cuts[c + 1]] - c * Gpc
        gslot = inv_orderp[c, gl >> 7]
        o = np.argsort(gslot, kind="stable")
        loc_o, gl_o, gs_o = loc[o], gl[o], gslot[o]
        startp = np.zeros(GB + 1, np.int64)
        np.add.at(startp, gs_o + 1, 1)
        startp = np.cumsum(startp)
        rank = np.arange(len(loc_o)) - startp[gs_o]
        pos = (offp[gs_o] + (rank >> 7)) * P + (rank & 127)
        pidx = np.zeros(Jp * P, np.int64)
        grel = np.full((P, Jp), -1.0, np.float32)
        pidx[pos] = slot2[c][loc_o]
        grel[pos & 127, pos >> 7] = (gl_o & 127).astype(np.float32)

        def slot_layout_i32(vals):
            a = np.zeros((P, len(vals) // P), np.int32)
            a[np.arange(len(vals)) & 127, np.arange(len(vals)) >> 7] = vals
            return np.ascontiguousarray(a)

        cores.append(dict(
            idx1w=_wrap_idx(idx1), idx1g=slot_layout_i32(idx1),
            dstl1=dstl1, norm1=norm1,
            idx2w=_wrap_idx(idx2), idx2g=slot_layout_i32(idx2g),
            dstl2=dstl2,
            dinv1col=dinv1col, dinv2col=dinv2col,
            sqdeg=sqdeg.reshape(1, NB2 * P),
            pidxw=_wrap_idx(pidx), grel=grel,
            _idx1_all=idx1, _idx2g_all=idx2g, _pidx_all=pidx,
        ))

    meta = dict(NB1=NB1, NB2=NB2, J1=J1, J2=J2, Jp=Jp, GB=GB, Gpc=Gpc,
                Lpad=Lpad,
                K1_slot=tuple(int(x) for x in K1_slot),
                K2q_slot=tuple(tuple(int(x) for x in r) for r in K2q_slot),
                Kp_slot=tuple(int(x) for x in Kp_slot),
                sb1_info=tuple((tuple(b), js, c0) for b, js, c0 in sb1_info),
                sb2_info=tuple(
                    (tuple(b), js, c0, tuple(qr)) for b, js, c0, qr in sb2_info),
                off1=tuple(int(x) for x in off1),
                sb1_split=tuple(int(x) for x in sb1_split),
                sb2_split=tuple(int(x) for x in sb2_split),
                col2=tuple(tuple(int(x) for x in r) for r in col2),
                offp=tuple(int(x) for x in offp))
    aux = dict(slot1=slot1, slot2=slot2, inv_orderp=inv_orderp, cuts=cuts,
               Ls=Ls)
    return cores, meta, aux


# ------------------------------------------------------------ launch AB


def build_ab(meta, vpad, has_b2):
    NB1, J1, Lpad = meta["NB1"], meta["J1"], meta["Lpad"]
    K1_slot, sb1_info, off1 = meta["K1_slot"], meta["sb1_info"], meta["off1"]
    sb1_split = meta["sb1_split"]
    nc = bacc.Bacc("TRN2", target_bir_lowering=False, debug=False,
                   num_devices=NCORES)
    embW1 = nc.dram_tensor("embW1", [vpad, HID], F16, kind="ExternalInput")
    idx1w = nc.dram_tensor("idx1w", [P, J1 * 8], I16, kind="ExternalInput")
    idx1g = nc.dram_tensor("idx1g", [P, J1], I32, kind="ExternalInput")
    dstl1 = nc.dram_tensor("dstl1", [P, J1], F32, kind="ExternalInput")
    norm1 = nc.dram_tensor("norm1", [P, J1], F32, kind="ExternalInput")
    W2 = nc.dram_tensor("W2", [HID, HID], F16, kind="ExternalInput")
    b1 = nc.dram_tensor("b1", [HID, 1], F32, kind="ExternalInput")
    dinv1col = nc.dram_tensor("dinv1col", [P, NB1], F32, kind="ExternalInput")
    iota = nc.dram_tensor("iota", [P, P], F16, kind="ExternalInput")
    h2 = nc.dram_tensor("h2", [Lpad, HID], F16, kind="ExternalOutput")

    from contextlib import ExitStack
    with tile.TileContext(nc) as tc, ExitStack() as ctx:
        const_p = ctx.enter_context(tc.tile_pool(name="constp", bufs=1))
        W2_sb = const_p.tile([HID, HID], F16)
        nc.sync.dma_start(W2_sb[:, :], W2[:, :])
        b1_sb = const_p.tile([HID, 1], F32)
        nc.sync.dma_start(b1_sb[:, :], b1[:, :])
        iota_sb = const_p.tile([P, P], F16)
        nc.sync.dma_start(iota_sb[:, :], iota[:, :])
        dinv_sb = const_p.tile([P, NB1], F32)
        nc.sync.dma_start(dinv_sb[:, :], dinv1col[:, :])

        idx_p = ctx.enter_context(tc.tile_pool(name="idxp", bufs=2))
        msg_p = ctx.enter_context(tc.tile_pool(name="msgp", bufs=2))
        mt_p = ctx.enter_context(tc.tile_pool(name="mtp", bufs=4))
        xo_p = ctx.enter_context(tc.tile_pool(name="xop", bufs=3))
        agg_p = ctx.enter_context(tc.tile_pool(name="aggps", bufs=2, space="PSUM"))
        h2_p = ctx.enter_context(tc.tile_pool(name="h2ps", bufs=2, space="PSUM"))

        from concourse.bass import IndirectOffsetOnAxis
        for isb, (blocks, Js, col0) in enumerate(sb1_info):
            nG = sb1_split[isb] - col0
            dstl_t = idx_p.tile([P, Js], F32, tag="dstl")
            nc.sync.dma_start(dstl_t[:, :], dstl1[:, col0:col0 + Js])
            norm_t = idx_p.tile([P, Js], F32, tag="norm")
            nc.sync.dma_start(norm_t[:, :], norm1[:, col0:col0 + Js])

            msg_t = msg_p.tile([P, Js, HID], F16, tag="msg")
            if nG > 0:
                idx_t = idx_p.tile([P, nG * 8], I16, tag="idx")
                nc.sync.dma_start(idx_t[:, :], idx1w[:, col0 * 8:(col0 + nG) * 8])
                nc.gpsimd.dma_gather(
                    msg_t[:, 0:nG, :], embW1[:, :], idx_t[:, :],
                    num_idxs=nG * P, num_idxs_reg=nG * P, elem_size=HID,
                    single_packet=False)
            if Js - nG > 0:
                nI = Js - nG
                idxg_t = idx_p.tile([P, nI], I32, tag="idxg")
                nc.sync.dma_start(idxg_t[:, :], idx1g[:, col0 + nG:col0 + Js])
                nc.gpsimd.indirect_dma_start(
                    out=msg_t[:, nG:Js, :], out_offset=None, in_=embW1[:, :],
                    in_offset=IndirectOffsetOnAxis(ap=idxg_t[:, 0:nI], axis=0))

            for b in blocks:
                K = K1_slot[b]
                agg = agg_p.tile([P, P], F32, tag="agg")
                for k in range(K):
                    j = off1[b] - col0 + k
                    mt = mt_p.tile([P, P], F16, tag="mt")
                    nc.vector.tensor_scalar(
                        out=mt[:, :], in0=iota_sb[:, :],
                        scalar1=dstl_t[:, j:j + 1], scalar2=norm_t[:, j:j + 1],
                        op0=OP.is_equal, op1=OP.mult)
                    nc.tensor.matmul(agg[:, :], lhsT=msg_t[:, j, :],
                                     rhs=mt[:, :], start=(k == 0),
                                     stop=(k == K - 1))
                # agg is [h, d]; relu + bias(per-partition h)
                xT = xo_p.tile([P, P], F16, tag="xT")
                nc.scalar.activation(xT[:, :], agg[:, :], AF.Relu,
                                     bias=b1_sb[:, :])
                h2ps = h2_p.tile([P, P], F32, tag="h2ps")
                nc.tensor.matmul(h2ps[:, :], lhsT=xT[:, :], rhs=W2_sb[:, :],
                                 start=True, stop=True)
                h2sb = xo_p.tile([P, P], F16, tag="h2sb")
                nc.scalar.activation(h2sb[:, :], h2ps[:, :], AF.Copy,
                                     scale=dinv_sb[:, b:b + 1])
                nc.sync.dma_start(h2[b * P:(b + 1) * P, :], h2sb[:, :])
    nc.compile()
    return nc


# ------------------------------------------------------------ launch C


def build_c(meta, has_b2):
    NB2, J2, Jp, GB, Lpad = (meta["NB2"], meta["J2"], meta["Jp"], meta["GB"],
                             meta["Lpad"])
    K2q_slot, sb2_info, col2 = meta["K2q_slot"], meta["sb2_info"], meta["col2"]
    sb2_split = meta["sb2_split"]
    Kp_slot, offp = meta["Kp_slot"], meta["offp"]
    TBL = NCORES * Lpad
    QROWS = 2 * Lpad
    nc = bacc.Bacc("TRN2", target_bir_lowering=False, debug=False,
                   num_devices=NCORES)
    h2tab = nc.dram_tensor("h2tab", [TBL, HID], F16, kind="ExternalInput")
    idx2w = nc.dram_tensor("idx2w", [P, J2 * 8], I16, kind="ExternalInput")
    idx2g = nc.dram_tensor("idx2g", [P, J2], I32, kind="ExternalInput")
    dstl2 = nc.dram_tensor("dstl2", [P, J2], F32, kind="ExternalInput")
    sqdeg = nc.dram_tensor("sqdeg", [1, NB2 * P], F16, kind="ExternalInput")
    b2row = nc.dram_tensor("b2row", [1, HID], F16, kind="ExternalInput")
    dinv2col = nc.dram_tensor("dinv2col", [P, NB2], F32, kind="ExternalInput")
    iota = nc.dram_tensor("iota", [P, P], F16, kind="ExternalInput")
    pidxw = nc.dram_tensor("pidxw", [P, Jp * 8], I16, kind="ExternalInput")
    grel = nc.dram_tensor("grel", [P, Jp], F32, kind="ExternalInput")
    Wout = nc.dram_tensor("Wout", [HID, NCLS], F16, kind="ExternalInput")
    bout = nc.dram_tensor("bout", [1, NCLS], F32, kind="ExternalInput")
    out = nc.dram_tensor("out", [GB * P, NCLS], F32, kind="ExternalOutput")

    from contextlib import ExitStack
    with tile.TileContext(nc) as tc, ExitStack() as ctx:
        const_p = ctx.enter_context(tc.tile_pool(name="constp", bufs=1))
        dram_p = ctx.enter_context(tc.tile_pool(name="dramp", bufs=1,
                                                space="DRAM"))
        iota_sb = const_p.tile([P, P], F16)
        nc.sync.dma_start(iota_sb[:, :], iota[:, :])
        dinv_sb = const_p.tile([P, NB2], F32)
        nc.sync.dma_start(dinv_sb[:, :], dinv2col[:, :])
        Wout_sb = const_p.tile([HID, NCLS], F16)
        nc.sync.dma_start(Wout_sb[:, :], Wout[:, :])
        bout_sb = const_p.tile([1, NCLS], F32)
        nc.sync.dma_start(bout_sb[:, :], bout[:, :])
        bout_bc = const_p.tile([P, NCLS], F32)
        nc.gpsimd.partition_broadcast(bout_bc[:, :], bout_sb[:, :])
        ones_sb = const_p.tile([P, 1], F16)
        nc.vector.memset(ones_sb[:, :], 1.0)
        ident = nc.dram_tensor("ident", [P, P], F16, kind="ExternalInput")
        ident_sb = const_p.tile([P, P], F16)
        nc.sync.dma_start(ident_sb[:, :], ident[:, :])
        if has_b2:
            sq_sb = const_p.tile([1, NB2 * P], F16)
            nc.sync.dma_start(sq_sb[:, :], sqdeg[:, :])
            b2_sb = const_p.tile([1, HID], F16)
            nc.sync.dma_start(b2_sb[:, :], b2row[:, :])

        x3d = dram_p.tile([Lpad, HID], F16)

        idx_p = ctx.enter_context(tc.tile_pool(name="idxp", bufs=2))
        msg_p = ctx.enter_context(tc.tile_pool(name="msgp", bufs=2))
        mt_p = ctx.enter_context(tc.tile_pool(name="mtp", bufs=4))
        xo_p = ctx.enter_context(tc.tile_pool(name="xop", bufs=3))
        agg_p = ctx.enter_context(tc.tile_pool(name="aggps", bufs=2, space="PSUM"))

        from concourse.bass import IndirectOffsetOnAxis
        self_p = ctx.enter_context(tc.tile_pool(name="selfp", bufs=3))
        for isb, (blocks, Js, col0, qruns) in enumerate(sb2_info):
            split = sb2_split[isb]
            dstl_t = idx_p.tile([P, Js], F32, tag="dstl")
            nc.sync.dma_start(dstl_t[:, :], dstl2[:, col0:col0 + Js])

            msg_t = msg_p.tile([P, Js, HID], F16, tag="msg")
            nG = split - col0
            if nG > 0:
                idx_t = idx_p.tile([P, nG * 8], I16, tag="idx")
                nc.sync.dma_start(idx_t[:, :], idx2w[:, col0 * 8:(col0 + nG) * 8])
            for q, c0, c1 in qruns:
                c1g = min(c1, split)
                if c1g <= c0:
                    continue
                nq = c1g - c0
                r0 = c0 - col0
                nc.gpsimd.dma_gather(
                    msg_t[:, r0:r0 + nq, :],
                    h2tab[q * QROWS:(q + 1) * QROWS, :],
                    idx_t[:, r0 * 8:(r0 + nq) * 8],
                    num_idxs=nq * P, num_idxs_reg=nq * P, elem_size=HID,
                    single_packet=False)
            if col0 + Js - split > 0:
                nI = col0 + Js - split
                r0 = split - col0
                idxg_t = idx_p.tile([P, nI], I32, tag="idxg")
                nc.sync.dma_start(idxg_t[:, :], idx2g[:, split:split + nI])
                nc.gpsimd.indirect_dma_start(
                    out=msg_t[:, r0:r0 + nI, :], out_offset=None,
                    in_=h2tab[:, :],
                    in_offset=IndirectOffsetOnAxis(ap=idxg_t[:, 0:nI], axis=0))

            for b in blocks:
                selfmsg = self_p.tile([P, P], F16, tag="selfmsg")
                nc.sync.dma_start(selfmsg[:, :], h2tab[b * P:(b + 1) * P, :])
                agg = agg_p.tile([P, P], F32, tag="agg")
                nchunks = sum(K2q_slot[b]) + 1
                done = 1
                nc.tensor.matmul(agg[:, :], lhsT=ident_sb[:, :],
                                 rhs=selfmsg[:, :], start=True,
                                 stop=(done == nchunks and not has_b2))
                if has_b2:
                    nc.tensor.matmul(
                        agg[:, :], lhsT=sq_sb[:, b * P:(b + 1) * P],
                        rhs=b2_sb[:, :], start=False,
                        stop=(sum(K2q_slot[b]) == 0))
                for q in range(4):
                    for k in range(K2q_slot[b][q]):
                        j = col2[b][q] - col0 + k
                        done += 1
                        mt = mt_p.tile([P, P], F16, tag="mt")
                        nc.vector.tensor_scalar(
                            out=mt[:, :], in0=iota_sb[:, :],
                            scalar1=dstl_t[:, j:j + 1], scalar2=None,
                            op0=OP.is_equal)
                        nc.tensor.matmul(agg[:, :], lhsT=mt[:, :],
                                         rhs=msg_t[:, j, :], start=False,
                                         stop=(done == nchunks))
                # agg is [d, h]; x3 = relu(dinv_d * agg (+ b2))
                x3sb = xo_p.tile([P, P], F16, tag="x3sb")
                nc.scalar.activation(x3sb[:, :], agg[:, :], AF.Relu,
                                     scale=dinv_sb[:, b:b + 1])
                nc.sync.dma_start(x3d[b * P:(b + 1) * P, :], x3sb[:, :])

        # ---------------- pool + head
        pool_p = ctx.enter_context(tc.tile_pool(name="poolp", bufs=2))
        pps = ctx.enter_context(tc.tile_pool(name="poolps", bufs=2, space="PSUM"))
        cps = ctx.enter_context(tc.tile_pool(name="cntps", bufs=2, space="PSUM"))
        for g in range(GB):
            Kp = Kp_slot[g]
            c0 = offp[g]
            pidx_t = pool_p.tile([P, Kp * 8], I16, tag="pidx")
            nc.sync.dma_start(pidx_t[:, :], pidxw[:, c0 * 8:(c0 + Kp) * 8])
            grel_t = pool_p.tile([P, Kp], F32, tag="grel")
            nc.sync.dma_start(grel_t[:, :], grel[:, c0:c0 + Kp])
            x3p_t = pool_p.tile([P, Kp, HID], F16, tag="x3p")
            nc.gpsimd.dma_gather(
                x3p_t[:, :, :], x3d[:, :], pidx_t[:, :],
                num_idxs=Kp * P, num_idxs_reg=Kp * P, elem_size=HID, single_packet=False)
            poolps = pps.tile([P, P], F32, tag="poolps")
            cntps = cps.tile([P, 1], F32, tag="cntps")
            for k in range(Kp):
                mp = pool_p.tile([P, P], F16, tag="mp")
                nc.vector.tensor_scalar(
                    out=mp[:, :], in0=iota_sb[:, :],
                    scalar1=grel_t[:, k:k + 1], scalar2=None, op0=OP.is_equal)
                nc.tensor.matmul(poolps[:, :], lhsT=x3p_t[:, k, :],
                                 rhs=mp[:, :], start=(k == 0),
                                 stop=(k == Kp - 1))
                nc.tensor.matmul(cntps[:, :], lhsT=mp[:, :], rhs=ones_sb[:, :],
                                 start=(k == 0), stop=(k == Kp - 1))
            cntm = pool_p.tile([P, 1], F32, tag="cntm")
            nc.vector.tensor_scalar_max(cntm[:, :], cntps[:, :], 1.0)
            rec = pool_p.tile([P, 1], F32, tag="rec")
            nc.vector.reciprocal(rec[:, :], cntm[:, :])
            poolT = pool_p.tile([P, P], F16, tag="poolT")
            nc.scalar.activation(poolT[:, :], poolps[:, :], AF.Copy)
            headps = cps.tile([P, NCLS], F32, tag="headps")
            nc.tensor.matmul(headps[:, :], lhsT=poolT[:, :], rhs=Wout_sb[:, :],
                             start=True, stop=True)
            osb = pool_p.tile([P, NCLS], F32, tag="osb")
            nc.vector.tensor_scalar(out=osb[:, :], in0=headps[:, :],
                                    scalar1=rec[:, :], scalar2=None,
                                    op0=OP.mult)
            osb2 = pool_p.tile([P, NCLS], F32, tag="osb2")
            nc.vector.tensor_tensor(out=osb2[:, :], in0=osb[:, :],
                                    in1=bout_bc[:, :], op=OP.add)
            nc.sync.dma_start(out[g * P:(g + 1) * P, :], osb2[:, :])
    nc.compile()
    return nc


# ---------------------------------------------------------------- entry point


_CACHE = {}
LAST_TIMES = {}


def kernel(node_ids, edge_index, batch, embed, W1, b1, W2, b2, Wout, bout,
           n_graphs=8192):
    from concourse import bass_utils
    cores, meta, aux = _prep(node_ids, edge_index, batch, n_graphs)
    NB1, NB2, GB, Gpc, Lpad = (meta["NB1"], meta["NB2"], meta["GB"],
                               meta["Gpc"], meta["Lpad"])

    V = embed.shape[0]
    vpad = _ceil(V, P) * P
    embW1 = np.zeros((vpad, HID), np.float16)
    embW1[:V] = (np.asarray(embed, np.float64)
                 @ np.asarray(W1, np.float64)).astype(np.float16)
    iota = np.tile(np.arange(P, dtype=np.float16), (P, 1))
    has_b2 = bool(np.any(np.asarray(b2) != 0))

    key = ("ab3", vpad, has_b2) + tuple(
        meta[k] for k in ("NB1", "J1", "K1_slot", "sb1_info", "off1", "Lpad",
                          "sb1_split"))
    if key not in _CACHE:
        _CACHE[key] = build_ab(meta, vpad, has_b2)
    nc_ab = _CACHE[key]
    sh = dict(embW1=embW1, W2=np.asarray(W2, np.float16),
              b1=np.asarray(b1, np.float32).reshape(HID, 1), iota=iota)
    in_ab = [dict(sh, idx1w=c["idx1w"], idx1g=c["idx1g"], dstl1=c["dstl1"],
                  norm1=c["norm1"], dinv1col=c["dinv1col"]) for c in cores]
    res_ab = bass_utils.run_bass_kernel_spmd(nc_ab, in_ab, list(range(NCORES)))
    LAST_TIMES["ab"] = res_ab.exec_time_ns

    h2all = np.stack([np.asarray(res_ab.results[c]["h2"], np.float16)
                      for c in range(NCORES)])  # [8, Lpad, HID]
    key2 = ("c3", has_b2) + tuple(
        meta[k] for k in ("NB2", "J2", "Jp", "GB", "K2q_slot", "sb2_info",
                          "col2", "Kp_slot", "offp", "Lpad", "sb2_split"))
    if key2 not in _CACHE:
        _CACHE[key2] = build_c(meta, has_b2)
    nc_c = _CACHE[key2]
    shc = dict(iota=iota, Wout=np.asarray(Wout, np.float16),
               bout=np.asarray(bout, np.float32).reshape(1, NCLS),
               b2row=np.asarray(b2, np.float16).reshape(1, HID),
               ident=np.eye(P, dtype=np.float16))
    in_c = []
    for c in range(NCORES):
        rot = np.roll(np.arange(NCORES), -c)
        h2tab = np.ascontiguousarray(
            h2all[rot].reshape(NCORES * Lpad, HID))
        in_c.append(dict(shc, h2tab=h2tab, idx2w=cores[c]["idx2w"],
                         idx2g=cores[c]["idx2g"],
                         dstl2=cores[c]["dstl2"], sqdeg=cores[c]["sqdeg"],
                         dinv2col=cores[c]["dinv2col"],
                         pidxw=cores[c]["pidxw"], grel=cores[c]["grel"]))
    res_c = bass_utils.run_bass_kernel_spmd(nc_c, in_c, list(range(NCORES)))
    LAST_TIMES["c"] = res_c.exec_time_ns

    out = np.empty((n_graphs, NCLS), np.float32)
    for c in range(NCORES):
        o = np.asarray(res_c.results[c]["out"], np.float32)  # [GB*P, NCLS]
        g = np.arange(Gpc)
        rows = aux["inv_orderp"][c, g >> 7] * P + (g & 127)
        out[c * Gpc:(c + 1) * Gpc] = o[rows]
    return out
